# revision 30
# baseline (speedup 1.0000x reference)
"""Causal multi-head attention on 8 TRN2 NeuronCores.

Problem: x[4,2048,1024], w_attn[1024,3072], w_proj[1024,1024],
16 heads x 64 dim, causal softmax(QK^T/8)V then output projection.

Sharding: 4-way batch x 2-way head-half. Core c handles batch c//2 and
heads (c%2)*8 .. (c%2)*8+8. Each core computes a partial y^T (its head
half's contribution to the output projection); the host sums the two
partials per batch and transposes.

Per-core layout strategy (all matmuls bf16, cost ~= moving-free-dim):
 - host feeds x^T [1024, 2048] (c_in-major)
 - QKV projection: Q^T,K^T computed feature-major [512, T]; V computed
   token-major [T, 512] (so no on-device transposes anywhere)
 - attention computed transposed: S^T[k,q] = (K^T).T-slices @ Q^T with
   K=64 contraction as two 64-row-base matmuls (head parity 0/64)
 - P = exp(S^T/8) on ACT straight out of PSUM (bf16 output); causal
   handling: sub-diagonal chunks skipped, diagonal chunks computed and
   accumulated only on their valid column range [off:512] (no left
   zero-fill needed), the 128-wide diagonal strip masked on DVE
 - PV: O^T[d,q] accumulated over key tiles with stationary [V_h | 1]
   (65 cols); PSUM row 64 carries the softmax denominators for free
 - normalize: DVE reciprocal into row0 of a bci tile, copy to row 32,
   stream_shuffle broadcast, then one scalar_tensor_tensor that reads
   O^T straight from PSUM and multiplies by the broadcast reciprocal
   (evacuation and normalization fused, ACT untouched)
 - projection: y^T partial = w_proj_slice.T-rows @ O^T; C-chunk m-tiles
   are queued and interleaved into the NEXT unit's S-loop so their PE
   work fills exp-bound gaps; evac to SBUF on DVE, then DMA out
 - software pipeline: S^T/exp of unit i interleaved with PV of unit
   i-1 so the in-order PE stream always has matmul work while ACT
   chews through the exps
 - PSUM budget: S 3 banks + po 4 banks (lazy alloc) + C 1 bank = 8
"""

import numpy as np
from contextlib import ExitStack

import concourse.bass as bass
import concourse.tile as tile
from concourse import bacc, mybir
from concourse.bass_utils import run_bass_kernel_spmd

f32 = mybir.dt.float32
bf16 = mybir.dt.bfloat16
EXP = mybir.ActivationFunctionType.Exp
MUL = mybir.AluOpType.mult

B, T, C = 4, 2048, 1024
N_HEAD, HD = 16, 64
HPC = 8            # heads per core
FS = HPC * HD      # 512: per-core feature slice for each of q/k/v
NPAIR = HPC // 2   # 4 head pairs
SCALE = 1.0 / 8.0  # 1/sqrt(64)
N_CORES = 8


def build_nc(tpc=T, loop_n=1, dyn_loop=0, stages='ABC'):
    """Build the single-core Bass program (SPMD: same program all cores)."""
    nck = C // 128          # 8 c_in tiles
    nkt = tpc // 128        # key tiles
    nqc = tpc // 512        # query chunks (512 wide)
    nmt = C // 128          # 8 output-channel tiles

    nc = bacc.Bacc("TRN2", target_bir_lowering=False)
    xt = nc.dram_tensor("xt", [C, tpc], bf16, kind="ExternalInput")
    wq = nc.dram_tensor("wq", [C, FS], bf16, kind="ExternalInput")
    wk = nc.dram_tensor("wk", [C, FS], bf16, kind="ExternalInput")
    wv = nc.dram_tensor("wv", [C, FS], bf16, kind="ExternalInput")
    wp = nc.dram_tensor("wp", [FS, C], bf16, kind="ExternalInput")
    mk = nc.dram_tensor("mk", [128, 128], bf16, kind="ExternalInput")
    yt = nc.dram_tensor("yt", [C, tpc], f32, kind="ExternalOutput")

    with tile.TileContext(nc) as tc, ExitStack() as _dl:
     if dyn_loop:
        _dl.enter_context(tc.For_i(0, dyn_loop, 1))
     for _rep in range(loop_n):
      with ExitStack() as stk:
            # tensors that live across stages
            persist = stk.enter_context(tc.tile_pool(name="persist", bufs=1))
            qT = [persist.tile([128, tpc], bf16, tag=f"qT{p}", name=f"qT{p}") for p in range(NPAIR)]
            kT = [persist.tile([128, tpc], bf16, tag=f"kT{p}", name=f"kT{p}") for p in range(NPAIR)]
            # vhat[kt]: [128 keys, 8 heads, 64 dims + ones column]
            vhat = [persist.tile([128, HPC, 65], bf16, tag=f"vh{t}", name=f"vh{t}")
                    for t in range(nkt)]
            mkt = persist.tile([128, 128], bf16, tag="mk")
            nc.sync.dma_start(out=mkt, in_=mk[:, :])
            ones_f = persist.tile([128, HPC], f32, tag="ones")
            nc.vector.memset(ones_f[:, :], 1.0)
            # [1,64] ones: stationary for the K=1 broadcast matmul in norm
            ones_bc = persist.tile([1, 64], bf16, tag="onesbc")
            nc.vector.memset(ones_bc[:, :], 1.0)

            # ---------------- Stage A: Q/K projection ----------------
            xa = stk.enter_context(tc.tile_pool(name="xa", bufs=1))
            wb = stk.enter_context(tc.tile_pool(name="wb", bufs=1))
            with tc.tile_pool(name="wa", bufs=1) as wa, \
                 tc.tile_pool(name="psa", bufs=4, space="PSUM") as psa:
                # x^T in 3 chunked DMAs (SP dma_start issue is ~1.6us each;
                # batching beats 8 serial issues)
                xall = xa.tile([128, nck, tpc], bf16, tag="x")
                xsrc = xt.rearrange("(a p) f -> p a f", p=128)
                for i in range(nck):   # 8 DMAs -> 8 parallel HW queues
                    nc.sync.dma_start(out=xall[:, i, :], in_=xsrc[:, i, :])
                xts = [xall[:, i, :] for i in range(nck)]

                # Q^T / K^T feature-major: out[feat, tok]; one DMA per weight
                wts = {}
                for nm, wsrc in (("q", wq), ("k", wk)):
                    wt = wa.tile([128, nck, FS], bf16, tag=f"w{nm}", name=f"wt{nm}")
                    nc.sync.dma_start(
                        out=wt, in_=wsrc.rearrange("(a p) f -> p a f", p=128))
                    wts[nm] = wt
                nw = tpc // 512            # psum bank limits N*f32 to 512
                wd = tpc // nw
                for dst, nm in ((qT, "q"), (kT, "k")):
                    for m in range(NPAIR):
                        wt = wts[nm]
                        for n in range(nw):
                            ps = psa.tile([128, wd], f32, tag="ps")
                            for k in range(nck):
                                nc.tensor.matmul(
                                    ps[:, :], wt[:, k, m * 128:(m + 1) * 128],
                                    xts[k][:, n * wd:(n + 1) * wd],
                                    start=(k == 0), stop=(k == nck - 1))
                            nc.scalar.activation(
                                dst[m][:, n * wd:(n + 1) * wd], ps[:, :],
                                mybir.ActivationFunctionType.Copy)

                # V weights loaded here; V compute is interleaved into B
                wvt = wb.tile([128, nck, FS], bf16, tag="wv")
                nc.sync.dma_start(out=wvt, in_=wv.rearrange("(a p) f -> p a f", p=128))

            # ---------------- Stages B+C ----------------
            otp = stk.enter_context(tc.tile_pool(name="ot", bufs=1))
            oT = [otp.tile([128, tpc], bf16, tag=f"oT{p}", name=f"oT{p}") for p in range(NPAIR)]

            wc = stk.enter_context(tc.tile_pool(name="wc", bufs=1))
            units = ([(p, qc) for qc in range(nqc) for p in range(NPAIR)]
                     if ('B' in stages or 'S' in stages or 'P' in stages)
                     else [])
            s_only = ('B' not in stages and 'P' not in stages)
            do_norm = 'B' in stages      # 'P': S+PV but no norm/C
            wpt = None
            if 'C' in stages and units:
                wpt = wc.tile([128, NPAIR, C], bf16, tag="wp", name="wpt")
                nc.sync.dma_start(
                    out=wpt, in_=wp.rearrange("(a p) f -> p a f", p=128))

            with tc.tile_pool(name="pp", bufs=20) as pp, \
                 tc.tile_pool(name="rp", bufs=4) as rp, \
                 tc.tile_pool(name="ev", bufs=4) as ev, \
                 tc.tile_pool(name="psB", bufs=3, space="PSUM") as psB:
                # qc-major unit order (pairs inner) so all four pairs of a
                # query chunk finish together

                pend_c = []

                def emit_c_mtile(qc, m):
                    ps = psB.tile([128, 512], f32, tag="c", bufs=2, name="cps")
                    for j in range(NPAIR):
                        nc.tensor.matmul(
                            ps[:, :], wpt[:, j, m * 128:(m + 1) * 128],
                            oT[j][:, qc * 512:(qc + 1) * 512],
                            start=(j == 0), stop=(j == NPAIR - 1))
                    sb = ev.tile([128, 512], f32, tag="sb", name="sb")
                    nc.vector.tensor_copy(sb[:, :], ps[:, :])
                    nc.sync.dma_start(
                        out=yt[m * 128:(m + 1) * 128,
                               qc * 512:(qc + 1) * 512],
                        in_=sb)

                vstate = {"t": 0}

                def emit_v_unit():
                    t = vstate["t"]
                    if t >= nkt:
                        return
                    vstate["t"] += 1
                    ps = psB.tile([128, FS], f32, tag="s", bufs=2, name="vps")
                    for k in range(nck):
                        nc.tensor.matmul(
                            ps[:, :], xts[k][:, t * 128:(t + 1) * 128],
                            wvt[:, k, :],
                            start=(k == 0), stop=(k == nck - 1))
                    nc.vector.tensor_copy(
                        vhat[t][:, :, 0:HD],
                        ps[:, :].rearrange("p (h d) -> p h d", h=HPC))
                    nc.vector.tensor_copy(vhat[t][:, :, HD], ones_f[:, :])

                def emit_s_step(st):
                    """Emit one S^T + exp + mask step; returns False when done.

                    Both head parities share one 2-bank PSUM tile [128,2,512]
                    so a single 1024-wide exp drains them (halves ACT
                    instruction overhead vs one exp per parity)."""
                    p, qc, kts, i = st["p"], st["qc"], st["kts"], st["i"]
                    if i >= len(kts):
                        return False
                    kt = kts[i]
                    ksl = slice(kt * 128, (kt + 1) * 128)
                    diag = (kt // 4 == qc)
                    off = 128 * (kt % 4) if diag else 0
                    qs2 = slice(qc * 512 + off, (qc + 1) * 512)
                    ps = psB.tile([128, 2, 512], f32, tag="s", bufs=2, name="s")
                    for par in range(2):   # head parity: partitions 0/64
                        row = slice(64 * par, 64 * par + 64)
                        nc.tensor.matmul(
                            ps[:, par, off:512], kT[p][row, ksl],
                            qT[p][row, qs2], start=True, stop=True)
                    pr = pp.tile([128, 2, 512], bf16, tag="P", name="P")
                    nc.scalar.activation(pr[:, :, off:512], ps[:, :, off:512],
                                         EXP, scale=SCALE)
                    if diag:  # mask the 128-wide diagonal strip (both pars)
                        nc.vector.tensor_mul(
                            pr[:, :, off:off + 128],
                            pr[:, :, off:off + 128],
                            mkt[:, None, :].broadcast_to([128, 2, 128]))
                    st["ptiles"].append((pr, off))
                    st["i"] += 1
                    return True

                def emit_pv_chunk(st, n=4):
                    """Emit up to n PV key tiles as same-PSUM-bank matmul
                    runs (all n for parity 0, then all n for parity 1).
                    Consecutive matmuls into one bank avoid the per-MM
                    bank-switch micro-idle penalty. Returns False when done."""
                    p, kts, j0 = st["p"], st["kts"], st["j"]
                    if j0 >= len(kts):
                        return False
                    if st["po"] is None:   # lazy: allocate at first PV chunk
                        st["po"] = [psB.tile([128, 512], f32, tag="po",
                                             bufs=2, name="po")
                                    for _ in range(2)]
                    j1 = min(j0 + n, len(kts))
                    for par in range(2):
                        for j in range(j0, j1):
                            kt = kts[j]
                            pr, off = st["ptiles"][j]
                            nc.tensor.matmul(
                                st["po"][par][0:65, off:512],
                                vhat[kt][:, 2 * p + par, :],
                                pr[:, par, off:512],
                                start=(kt == 0), stop=(kt == kts[-1]))
                    st["j"] = j1
                    return True

                def emit_norm(st):
                    """Fused evacuate+normalize: oT = po[0:64] * (1/po[64]).

                    The den row is evacuated by ACT (fast PSUM port, idle
                    capacity), partition-broadcast by a K=1 matmul against a
                    ones column (213ns on PE), and DVE only runs two
                    full-lane ops (reciprocal + fused multiply) — no
                    single-partition crawls or stream_shuffle."""
                    p, qc = st["p"], st["qc"]
                    qsl = slice(qc * 512, (qc + 1) * 512)
                    for par in range(2):
                        po = st["po"][par]
                        dsb = rp.tile([1, 512], bf16, tag="dsb", name="dsb")
                        nc.scalar.activation(
                            dsb[:, :], po[64:65, :],
                            mybir.ActivationFunctionType.Copy)
                        nc.tensor.matmul(po[64:128, :], ones_bc[0:1, :],
                                         dsb[0:1, :], start=True, stop=True)
                        rden = rp.tile([64, 512], f32, tag="rden", name="rden")
                        nc.vector.reciprocal(rden[:, :], po[64:128, :])
                        nc.vector.scalar_tensor_tensor(
                            out=oT[p][64 * par:64 * par + 64, qsl],
                            in0=po[0:64, :], scalar=1.0, in1=rden[:, :],
                            op0=MUL, op1=MUL)

                def new_state(p, qc):
                    kts = list(range(min(nkt, 4 * (qc + 1))))
                    return {"p": p, "qc": qc, "kts": kts, "i": 0, "j": 0,
                            "ptiles": [], "po": None}

                def retire(st):
                    while emit_pv_chunk(st):
                        pass
                    if do_norm:
                        emit_norm(st)
                        if 'C' in stages and st["p"] == NPAIR - 1:
                            pend_c.extend(
                                (st["qc"], m) for m in range(nmt))

                # software pipeline: S-phase of unit u interleaved with
                # PV-phase of unit u-1; when u-1's PV is exhausted it is
                # retired immediately (norm emitted early so its PSUM banks
                # recycle)
                if units:
                    for _ in range(4):   # PV of the first unit needs vhat[0..3]
                        emit_v_unit()
                prev = None
                for (p, qc) in units:
                    emit_v_unit()        # one V tile per unit until done
                    cur = new_state(p, qc)
                    more_s = True
                    while more_s:
                        more_s = emit_s_step(cur)
                        if s_only:
                            continue
                        if prev is not None and cur["i"] % 4 == 0:
                            emit_pv_chunk(prev)
                            if prev["j"] >= len(prev["kts"]):
                                retire(prev)
                                prev = None
                        elif pend_c and cur["i"] % 2 == 1:
                            emit_c_mtile(*pend_c.pop(0))
                    if s_only:
                        continue
                    if prev is not None:
                        retire(prev)
                        prev = None
                    prev = cur
                if prev is not None and not s_only:
                    retire(prev)
                for it in pend_c:
                    emit_c_mtile(*it)
    nc.compile()
    return nc


def _make_masks():
    import ml_dtypes
    k = np.arange(128)[:, None]
    q = np.arange(128)[None, :]
    return (q >= k).astype(ml_dtypes.bfloat16)


_NC_CACHE = {}


def _get_nc(tpc=T):
    if tpc not in _NC_CACHE:
        _NC_CACHE[tpc] = build_nc(tpc)
    return _NC_CACHE[tpc]


def make_in_maps(x, w_attn, w_proj):
    import ml_dtypes
    bf = ml_dtypes.bfloat16
    masks = _make_masks()
    in_maps = []
    for core in range(N_CORES):
        b, hh = core // 2, core % 2
        s = slice(hh * FS, (hh + 1) * FS)
        in_maps.append({
            "xt": np.ascontiguousarray(np.asarray(x[b]).T).astype(bf),
            "wq": np.ascontiguousarray(w_attn[:, s]).astype(bf),
            "wk": np.ascontiguousarray(w_attn[:, C:][:, s]).astype(bf),
            "wv": np.ascontiguousarray(w_attn[:, 2 * C:][:, s]).astype(bf),
            "wp": np.ascontiguousarray(w_proj[hh * FS:(hh + 1) * FS, :]).astype(bf),
            "mk": masks,
        })
    return in_maps


def kernel(x, w_attn, w_proj):
    nc = _get_nc(T)
    in_maps = make_in_maps(x, w_attn, w_proj)
    res = run_bass_kernel_spmd(nc, in_maps, list(range(N_CORES)))
    y = np.empty((B, T, C), np.float32)
    for b in range(B):
        yt = res.results[2 * b]["yt"] + res.results[2 * b + 1]["yt"]
        y[b] = yt.T
    return y


# revision 32
# speedup vs baseline: 1.1295x; 1.1295x over previous
"""Causal multi-head attention on 8 TRN2 NeuronCores.

Problem: x[4,2048,1024], w_attn[1024,3072], w_proj[1024,1024],
16 heads x 64 dim, causal softmax(QK^T/8)V then output projection.

Sharding: 4-way batch x 2-way head-half. Core c handles batch c//2 and
heads (c%2)*8 .. (c%2)*8+8. Each core computes a partial y^T (its head
half's contribution to the output projection); the host sums the two
partials per batch and transposes.

Per-core layout strategy (all matmuls bf16, cost ~= moving-free-dim):
 - host feeds x^T [1024, 2048] (c_in-major)
 - QKV projection: Q^T,K^T computed feature-major [512, T]; V computed
   token-major [T, 512] (so no on-device transposes anywhere)
 - attention computed transposed: S^T[k,q] = (K^T).T-slices @ Q^T with
   K=64 contraction as two 64-row-base matmuls (head parity 0/64)
 - P = exp(S^T/8) on ACT straight out of PSUM (bf16 output); causal
   handling: sub-diagonal chunks skipped, diagonal chunks computed and
   accumulated only on their valid column range [off:512] (no left
   zero-fill needed), the 128-wide diagonal strip masked on DVE
 - PV: O^T[d,q] accumulated over key tiles with stationary [V_h | 1]
   (65 cols); PSUM row 64 carries the softmax denominators for free
 - normalize: DVE reciprocal into row0 of a bci tile, copy to row 32,
   stream_shuffle broadcast, then one scalar_tensor_tensor that reads
   O^T straight from PSUM and multiplies by the broadcast reciprocal
   (evacuation and normalization fused, ACT untouched)
 - projection: y^T partial = w_proj_slice.T-rows @ O^T; C-chunk m-tiles
   are queued and interleaved into the NEXT unit's S-loop so their PE
   work fills exp-bound gaps; evac to SBUF on DVE, then DMA out
 - software pipeline: S^T/exp of unit i interleaved with PV of unit
   i-1 so the in-order PE stream always has matmul work while ACT
   chews through the exps
 - PSUM budget: S 3 banks + po 4 banks (lazy alloc) + C 1 bank = 8
"""

import numpy as np
from contextlib import ExitStack

import concourse.bass as bass
import concourse.tile as tile
from concourse import bacc, mybir
from concourse.bass_utils import run_bass_kernel_spmd

f32 = mybir.dt.float32
bf16 = mybir.dt.bfloat16
EXP = mybir.ActivationFunctionType.Exp
MUL = mybir.AluOpType.mult

B, T, C = 4, 2048, 1024
N_HEAD, HD = 16, 64
HPC = 8            # heads per core
FS = HPC * HD      # 512: per-core feature slice for each of q/k/v
NPAIR = HPC // 2   # 4 head pairs
SCALE = 1.0 / 8.0  # 1/sqrt(64)
N_CORES = 8


def build_nc(tpc=T, loop_n=1, dyn_loop=0, stages='ABC'):
    """Build the single-core Bass program (SPMD: same program all cores)."""
    nck = C // 128          # 8 c_in tiles
    nkt = tpc // 128        # key tiles
    nqc = tpc // 512        # query chunks (512 wide)
    nmt = C // 128          # 8 output-channel tiles

    nc = bacc.Bacc("TRN2", target_bir_lowering=False)
    xt = nc.dram_tensor("xt", [C, tpc], bf16, kind="ExternalInput")
    wq = nc.dram_tensor("wq", [C, FS], bf16, kind="ExternalInput")
    wk = nc.dram_tensor("wk", [C, FS], bf16, kind="ExternalInput")
    wv = nc.dram_tensor("wv", [C, FS], bf16, kind="ExternalInput")
    wp = nc.dram_tensor("wp", [FS, C], bf16, kind="ExternalInput")
    mk = nc.dram_tensor("mk", [128, 128], bf16, kind="ExternalInput")
    yt = nc.dram_tensor("yt", [C, tpc], f32, kind="ExternalOutput")

    with tile.TileContext(nc) as tc, ExitStack() as _dl:
     if dyn_loop:
        _dl.enter_context(tc.For_i(0, dyn_loop, 1))
     for _rep in range(loop_n):
      with ExitStack() as stk:
            # tensors that live across stages
            persist = stk.enter_context(tc.tile_pool(name="persist", bufs=1))
            qT = [persist.tile([128, tpc], bf16, tag=f"qT{p}", name=f"qT{p}") for p in range(NPAIR)]
            kT = [persist.tile([128, tpc], bf16, tag=f"kT{p}", name=f"kT{p}") for p in range(NPAIR)]
            # vhat[kt]: [128 keys, 8 heads, 64 dims + ones column]
            vhat = [persist.tile([128, HPC, 65], bf16, tag=f"vh{t}", name=f"vh{t}")
                    for t in range(nkt)]
            mkt = persist.tile([128, 128], bf16, tag="mk")
            nc.sync.dma_start(out=mkt, in_=mk[:, :])
            ones_f = persist.tile([128, HPC], f32, tag="ones")
            nc.vector.memset(ones_f[:, :], 1.0)
            # persistent shuffle inputs (per parity): only rows 0/32 rewritten
            bcis = []
            for bi in range(2):
                b_ = persist.tile([64, 512], bf16, tag=f"bci{bi}", name=f"bci{bi}")
                nc.vector.memset(b_[:, :], 0.0)
                bcis.append(b_)

            # ---------------- Stage A: Q/K projection ----------------
            xa = stk.enter_context(tc.tile_pool(name="xa", bufs=1))
            wb = stk.enter_context(tc.tile_pool(name="wb", bufs=1))
            with tc.tile_pool(name="wa", bufs=1) as wa, \
                 tc.tile_pool(name="psa", bufs=4, space="PSUM") as psa:
                # x^T in 3 chunked DMAs (SP dma_start issue is ~1.6us each;
                # batching beats 8 serial issues)
                xall = xa.tile([128, nck, tpc], bf16, tag="x")
                xsrc = xt.rearrange("(a p) f -> p a f", p=128)
                for i in range(nck):   # 8 DMAs -> 8 parallel HW queues
                    nc.sync.dma_start(out=xall[:, i, :], in_=xsrc[:, i, :])
                xts = [xall[:, i, :] for i in range(nck)]

                # Q^T / K^T feature-major: out[feat, tok]; one DMA per weight
                wts = {}
                for nm, wsrc in (("q", wq), ("k", wk)):
                    wt = wa.tile([128, nck, FS], bf16, tag=f"w{nm}", name=f"wt{nm}")
                    nc.sync.dma_start(
                        out=wt, in_=wsrc.rearrange("(a p) f -> p a f", p=128))
                    wts[nm] = wt
                nw = tpc // 512            # psum bank limits N*f32 to 512
                wd = tpc // nw
                for dst, nm in ((qT, "q"), (kT, "k")):
                    for m in range(NPAIR):
                        wt = wts[nm]
                        for n in range(nw):
                            ps = psa.tile([128, wd], f32, tag="ps")
                            for k in range(nck):
                                nc.tensor.matmul(
                                    ps[:, :], wt[:, k, m * 128:(m + 1) * 128],
                                    xts[k][:, n * wd:(n + 1) * wd],
                                    start=(k == 0), stop=(k == nck - 1))
                            nc.scalar.activation(
                                dst[m][:, n * wd:(n + 1) * wd], ps[:, :],
                                mybir.ActivationFunctionType.Copy)

                # V weights loaded here; V compute is interleaved into B
                wvt = wb.tile([128, nck, FS], bf16, tag="wv")
                nc.sync.dma_start(out=wvt, in_=wv.rearrange("(a p) f -> p a f", p=128))

            # ---------------- Stages B+C ----------------
            otp = stk.enter_context(tc.tile_pool(name="ot", bufs=1))
            oT = [otp.tile([128, tpc], bf16, tag=f"oT{p}", name=f"oT{p}") for p in range(NPAIR)]

            wc = stk.enter_context(tc.tile_pool(name="wc", bufs=1))
            units = ([(p, qc) for qc in range(nqc) for p in range(NPAIR)]
                     if ('B' in stages or 'S' in stages or 'P' in stages)
                     else [])
            s_only = ('B' not in stages and 'P' not in stages)
            do_norm = 'B' in stages      # 'P': S+PV but no norm/C
            wpt = None
            if 'C' in stages and units:
                wpt = wc.tile([128, NPAIR, C], bf16, tag="wp", name="wpt")
                nc.sync.dma_start(
                    out=wpt, in_=wp.rearrange("(a p) f -> p a f", p=128))

            with tc.tile_pool(name="pp", bufs=20) as pp, \
                 tc.tile_pool(name="rp", bufs=4) as rp, \
                 tc.tile_pool(name="ev", bufs=4) as ev, \
                 tc.tile_pool(name="psB", bufs=3, space="PSUM") as psB:
                # qc-major unit order (pairs inner) so all four pairs of a
                # query chunk finish together

                pend_c = []

                def emit_c_mtile(qc, m):
                    ps = psB.tile([128, 512], f32, tag="c", bufs=2, name="cps")
                    for j in range(NPAIR):
                        nc.tensor.matmul(
                            ps[:, :], wpt[:, j, m * 128:(m + 1) * 128],
                            oT[j][:, qc * 512:(qc + 1) * 512],
                            start=(j == 0), stop=(j == NPAIR - 1))
                    sb = ev.tile([128, 512], f32, tag="sb", name="sb")
                    nc.scalar.activation(sb[:, :], ps[:, :],
                                         mybir.ActivationFunctionType.Copy)
                    nc.sync.dma_start(
                        out=yt[m * 128:(m + 1) * 128,
                               qc * 512:(qc + 1) * 512],
                        in_=sb)

                vstate = {"t": 0}

                def emit_v_unit():
                    t = vstate["t"]
                    if t >= nkt:
                        return
                    vstate["t"] += 1
                    ps = psB.tile([128, FS], f32, tag="s", bufs=2, name="vps")
                    for k in range(nck):
                        nc.tensor.matmul(
                            ps[:, :], xts[k][:, t * 128:(t + 1) * 128],
                            wvt[:, k, :],
                            start=(k == 0), stop=(k == nck - 1))
                    nc.scalar.activation(
                        vhat[t][:, :, 0:HD],
                        ps[:, :].rearrange("p (h d) -> p h d", h=HPC),
                        mybir.ActivationFunctionType.Copy)
                    nc.vector.tensor_copy(vhat[t][:, :, HD], ones_f[:, :])

                def emit_s_step(st):
                    """Emit one S^T + exp + mask step; returns False when done.

                    Both head parities share one 2-bank PSUM tile [128,2,512]
                    so a single 1024-wide exp drains them (halves ACT
                    instruction overhead vs one exp per parity)."""
                    p, qc, kts, i = st["p"], st["qc"], st["kts"], st["i"]
                    if i >= len(kts):
                        return False
                    kt = kts[i]
                    ksl = slice(kt * 128, (kt + 1) * 128)
                    diag = (kt // 4 == qc)
                    off = 128 * (kt % 4) if diag else 0
                    qs2 = slice(qc * 512 + off, (qc + 1) * 512)
                    ps = psB.tile([128, 2, 512], f32, tag="s", bufs=2, name="s")
                    for par in range(2):   # head parity: partitions 0/64
                        row = slice(64 * par, 64 * par + 64)
                        nc.tensor.matmul(
                            ps[:, par, off:512], kT[p][row, ksl],
                            qT[p][row, qs2], start=True, stop=True)
                    pr = pp.tile([128, 2, 512], bf16, tag="P", name="P")
                    nc.scalar.activation(pr[:, :, off:512], ps[:, :, off:512],
                                         EXP, scale=SCALE)
                    if diag:  # mask the 128-wide diagonal strip (both pars)
                        nc.vector.tensor_mul(
                            pr[:, :, off:off + 128],
                            pr[:, :, off:off + 128],
                            mkt[:, None, :].broadcast_to([128, 2, 128]))
                    st["ptiles"].append((pr, off))
                    st["i"] += 1
                    return True

                def emit_pv_chunk(st, n=4):
                    """Emit up to n PV key tiles as same-PSUM-bank matmul
                    runs (all n for parity 0, then all n for parity 1).
                    Consecutive matmuls into one bank avoid the per-MM
                    bank-switch micro-idle penalty. Returns False when done."""
                    p, kts, j0 = st["p"], st["kts"], st["j"]
                    if j0 >= len(kts):
                        return False
                    if st["po"] is None:   # lazy: allocate at first PV chunk
                        st["po"] = [psB.tile([128, 512], f32, tag="po",
                                             bufs=2, name="po")
                                    for _ in range(2)]
                    j1 = min(j0 + n, len(kts))
                    for par in range(2):
                        for j in range(j0, j1):
                            kt = kts[j]
                            pr, off = st["ptiles"][j]
                            nc.tensor.matmul(
                                st["po"][par][0:65, off:512],
                                vhat[kt][:, 2 * p + par, :],
                                pr[:, par, off:512],
                                start=(kt == 0), stop=(kt == kts[-1]))
                    st["j"] = j1
                    return True

                def emit_norm(st):
                    """Fused evacuate+normalize: oT = po[0:64] * (1/po[64]).

                    The den row is evacuated by ACT (fast PSUM port, idle
                    capacity), partition-broadcast by a K=1 matmul against a
                    ones column (213ns on PE), and DVE only runs two
                    full-lane ops (reciprocal + fused multiply) — no
                    single-partition crawls or stream_shuffle."""
                    p, qc = st["p"], st["qc"]
                    qsl = slice(qc * 512, (qc + 1) * 512)
                    for par in range(2):
                        po = st["po"][par]
                        ob = rp.tile([65, 512], bf16, tag="ob", name="ob")
                        nc.scalar.activation(
                            ob[:, :], po[0:65, :],
                            mybir.ActivationFunctionType.Copy)
                        bci = bcis[par]
                        nc.vector.tensor_copy(bci[0:1, :], ob[64:65, :])
                        nc.vector.tensor_copy(bci[32:33, :], ob[64:65, :])
                        bc = rp.tile([64, 512], bf16, tag="bc", name="bc")
                        nc.vector.stream_shuffle(bc[:, :], bci[:, :], [0] * 32)
                        rden = rp.tile([64, 512], bf16, tag="rden", name="rden")
                        with nc.allow_low_precision(
                                reason="bf16 softmax denominators"):
                            nc.vector.reciprocal(rden[:, :], bc[:, :])
                        nc.vector.tensor_mul(
                            oT[p][64 * par:64 * par + 64, qsl],
                            ob[0:64, :], rden[:, :])

                def new_state(p, qc):
                    kts = list(range(min(nkt, 4 * (qc + 1))))
                    return {"p": p, "qc": qc, "kts": kts, "i": 0, "j": 0,
                            "ptiles": [], "po": None}

                def retire(st):
                    while emit_pv_chunk(st):
                        pass
                    if do_norm:
                        emit_norm(st)
                        if 'C' in stages and st["p"] == NPAIR - 1:
                            pend_c.extend(
                                (st["qc"], m) for m in range(nmt))

                # software pipeline: S-phase of unit u interleaved with
                # PV-phase of unit u-1; when u-1's PV is exhausted it is
                # retired immediately (norm emitted early so its PSUM banks
                # recycle)
                if units:
                    for _ in range(4):   # PV of the first unit needs vhat[0..3]
                        emit_v_unit()
                prev = None
                for (p, qc) in units:
                    emit_v_unit()        # one V tile per unit until done
                    cur = new_state(p, qc)
                    more_s = True
                    while more_s:
                        more_s = emit_s_step(cur)
                        if s_only:
                            continue
                        if prev is not None and cur["i"] % 4 == 0:
                            emit_pv_chunk(prev)
                            if prev["j"] >= len(prev["kts"]):
                                retire(prev)
                                prev = None
                        elif pend_c and cur["i"] % 2 == 1:
                            emit_c_mtile(*pend_c.pop(0))
                    if s_only:
                        continue
                    if prev is not None:
                        retire(prev)
                        prev = None
                    prev = cur
                if prev is not None and not s_only:
                    retire(prev)
                for it in pend_c:
                    emit_c_mtile(*it)
    nc.compile()
    return nc


def _make_masks():
    import ml_dtypes
    k = np.arange(128)[:, None]
    q = np.arange(128)[None, :]
    return (q >= k).astype(ml_dtypes.bfloat16)


_NC_CACHE = {}


def _get_nc(tpc=T):
    if tpc not in _NC_CACHE:
        _NC_CACHE[tpc] = build_nc(tpc)
    return _NC_CACHE[tpc]


def make_in_maps(x, w_attn, w_proj):
    import ml_dtypes
    bf = ml_dtypes.bfloat16
    masks = _make_masks()
    in_maps = []
    for core in range(N_CORES):
        b, hh = core // 2, core % 2
        s = slice(hh * FS, (hh + 1) * FS)
        in_maps.append({
            "xt": np.ascontiguousarray(np.asarray(x[b]).T).astype(bf),
            "wq": np.ascontiguousarray(w_attn[:, s]).astype(bf),
            "wk": np.ascontiguousarray(w_attn[:, C:][:, s]).astype(bf),
            "wv": np.ascontiguousarray(w_attn[:, 2 * C:][:, s]).astype(bf),
            "wp": np.ascontiguousarray(w_proj[hh * FS:(hh + 1) * FS, :]).astype(bf),
            "mk": masks,
        })
    return in_maps


def kernel(x, w_attn, w_proj):
    nc = _get_nc(T)
    in_maps = make_in_maps(x, w_attn, w_proj)
    res = run_bass_kernel_spmd(nc, in_maps, list(range(N_CORES)))
    y = np.empty((B, T, C), np.float32)
    for b in range(B):
        yt = res.results[2 * b]["yt"] + res.results[2 * b + 1]["yt"]
        y[b] = yt.T
    return y


# revision 33
# speedup vs baseline: 1.2454x; 1.1026x over previous
"""Causal multi-head attention on 8 TRN2 NeuronCores.

Problem: x[4,2048,1024], w_attn[1024,3072], w_proj[1024,1024],
16 heads x 64 dim, causal softmax(QK^T/8)V then output projection.

Sharding: 4-way batch x 2-way head-half. Core c handles batch c//2 and
heads (c%2)*8 .. (c%2)*8+8. Each core computes a partial y^T (its head
half's contribution to the output projection); the host sums the two
partials per batch and transposes.

Per-core layout strategy (all matmuls bf16, cost ~= moving-free-dim):
 - host feeds x^T [1024, 2048] (c_in-major)
 - QKV projection: Q^T,K^T computed feature-major [512, T]; V computed
   token-major [T, 512] (so no on-device transposes anywhere)
 - attention computed transposed: S^T[k,q] = (K^T).T-slices @ Q^T with
   K=64 contraction as two 64-row-base matmuls (head parity 0/64)
 - P = exp(S^T/8) on ACT straight out of PSUM (bf16 output); causal
   handling: sub-diagonal chunks skipped, diagonal chunks computed and
   accumulated only on their valid column range [off:512] (no left
   zero-fill needed), the 128-wide diagonal strip masked on DVE
 - PV: O^T[d,q] accumulated over key tiles with stationary [V_h | 1]
   (65 cols); PSUM row 64 carries the softmax denominators for free
 - normalize: DVE reciprocal into row0 of a bci tile, copy to row 32,
   stream_shuffle broadcast, then one scalar_tensor_tensor that reads
   O^T straight from PSUM and multiplies by the broadcast reciprocal
   (evacuation and normalization fused, ACT untouched)
 - projection: y^T partial = w_proj_slice.T-rows @ O^T; C-chunk m-tiles
   are queued and interleaved into the NEXT unit's S-loop so their PE
   work fills exp-bound gaps; evac to SBUF on DVE, then DMA out
 - software pipeline: S^T/exp of unit i interleaved with PV of unit
   i-1 so the in-order PE stream always has matmul work while ACT
   chews through the exps
 - PSUM budget: S 3 banks + po 4 banks (lazy alloc) + C 1 bank = 8
"""

import numpy as np
from contextlib import ExitStack

import concourse.bass as bass
import concourse.tile as tile
from concourse import bacc, mybir
from concourse.bass_utils import run_bass_kernel_spmd

f32 = mybir.dt.float32
bf16 = mybir.dt.bfloat16
EXP = mybir.ActivationFunctionType.Exp
MUL = mybir.AluOpType.mult

B, T, C = 4, 2048, 1024
N_HEAD, HD = 16, 64
HPC = 8            # heads per core
FS = HPC * HD      # 512: per-core feature slice for each of q/k/v
NPAIR = HPC // 2   # 4 head pairs
SCALE = 1.0 / 8.0  # 1/sqrt(64)
N_CORES = 8


def build_nc(tpc=T, loop_n=1, dyn_loop=0, stages='ABC'):
    """Build the single-core Bass program (SPMD: same program all cores)."""
    nck = C // 128          # 8 c_in tiles
    nkt = tpc // 128        # key tiles
    nqc = tpc // 512        # query chunks (512 wide)
    nmt = C // 128          # 8 output-channel tiles

    nc = bacc.Bacc("TRN2", target_bir_lowering=False)
    xt = nc.dram_tensor("xt", [C, tpc], bf16, kind="ExternalInput")
    wq = nc.dram_tensor("wq", [C, FS], bf16, kind="ExternalInput")
    wk = nc.dram_tensor("wk", [C, FS], bf16, kind="ExternalInput")
    wv = nc.dram_tensor("wv", [C, FS], bf16, kind="ExternalInput")
    wp = nc.dram_tensor("wp", [FS, C], bf16, kind="ExternalInput")
    mk = nc.dram_tensor("mk", [128, 128], bf16, kind="ExternalInput")
    yt = nc.dram_tensor("yt", [C, tpc], f32, kind="ExternalOutput")

    with tile.TileContext(nc) as tc, ExitStack() as _dl:
     if dyn_loop:
        _dl.enter_context(tc.For_i(0, dyn_loop, 1))
     for _rep in range(loop_n):
      with ExitStack() as stk:
            # tensors that live across stages
            persist = stk.enter_context(tc.tile_pool(name="persist", bufs=1))
            qT = [persist.tile([128, tpc], bf16, tag=f"qT{p}", name=f"qT{p}") for p in range(NPAIR)]
            kT = [persist.tile([128, tpc], bf16, tag=f"kT{p}", name=f"kT{p}") for p in range(NPAIR)]
            # vhat[kt]: [128 keys, 8 heads, 64 dims + ones column]
            vhat = [persist.tile([128, HPC, 65], bf16, tag=f"vh{t}", name=f"vh{t}")
                    for t in range(nkt)]
            mkt = persist.tile([128, 128], bf16, tag="mk")
            nc.sync.dma_start(out=mkt, in_=mk[:, :])
            ones_f = persist.tile([128, HPC], f32, tag="ones")
            nc.vector.memset(ones_f[:, :], 1.0)
            # persistent shuffle inputs (per parity): only rows 0/32 rewritten
            bcis = []
            for bi in range(2):
                b_ = persist.tile([64, 512], bf16, tag=f"bci{bi}", name=f"bci{bi}")
                nc.vector.memset(b_[:, :], 0.0)
                bcis.append(b_)

            # ---------------- Stage A: Q/K projection ----------------
            xa = stk.enter_context(tc.tile_pool(name="xa", bufs=1))
            wb = stk.enter_context(tc.tile_pool(name="wb", bufs=1))
            with tc.tile_pool(name="wa", bufs=1) as wa, \
                 tc.tile_pool(name="psa", bufs=4, space="PSUM") as psa:
                # x^T in 3 chunked DMAs (SP dma_start issue is ~1.6us each;
                # batching beats 8 serial issues)
                xall = xa.tile([128, nck, tpc], bf16, tag="x")
                xsrc = xt.rearrange("(a p) f -> p a f", p=128)
                for i in range(nck):   # 8 DMAs -> 8 parallel HW queues
                    nc.sync.dma_start(out=xall[:, i, :], in_=xsrc[:, i, :])
                xts = [xall[:, i, :] for i in range(nck)]

                # Q^T / K^T feature-major: out[feat, tok]; one DMA per weight
                wts = {}
                for nm, wsrc in (("q", wq), ("k", wk)):
                    wt = wa.tile([128, nck, FS], bf16, tag=f"w{nm}", name=f"wt{nm}")
                    nc.sync.dma_start(
                        out=wt, in_=wsrc.rearrange("(a p) f -> p a f", p=128))
                    wts[nm] = wt
                nw = tpc // 512            # psum bank limits N*f32 to 512
                wd = tpc // nw
                for dst, nm in ((qT, "q"), (kT, "k")):
                    for m in range(NPAIR):
                        wt = wts[nm]
                        for n in range(nw):
                            ps = psa.tile([128, wd], f32, tag="ps")
                            for k in range(nck):
                                nc.tensor.matmul(
                                    ps[:, :], wt[:, k, m * 128:(m + 1) * 128],
                                    xts[k][:, n * wd:(n + 1) * wd],
                                    start=(k == 0), stop=(k == nck - 1))
                            nc.scalar.activation(
                                dst[m][:, n * wd:(n + 1) * wd], ps[:, :],
                                mybir.ActivationFunctionType.Copy)

                # V weights loaded here; V compute is interleaved into B
                wvt = wb.tile([128, nck, FS], bf16, tag="wv")
                nc.sync.dma_start(out=wvt, in_=wv.rearrange("(a p) f -> p a f", p=128))

            # ---------------- Stages B+C ----------------
            otp = stk.enter_context(tc.tile_pool(name="ot", bufs=1))
            oT = [otp.tile([128, tpc], bf16, tag=f"oT{p}", name=f"oT{p}") for p in range(NPAIR)]

            wc = stk.enter_context(tc.tile_pool(name="wc", bufs=1))
            units = ([(p, qc) for qc in range(nqc) for p in range(NPAIR)]
                     if ('B' in stages or 'S' in stages or 'P' in stages)
                     else [])
            s_only = ('B' not in stages and 'P' not in stages)
            do_norm = 'B' in stages      # 'P': S+PV but no norm/C
            wpt = None
            if 'C' in stages and units:
                wpt = wc.tile([128, NPAIR, C], bf16, tag="wp", name="wpt")
                nc.sync.dma_start(
                    out=wpt, in_=wp.rearrange("(a p) f -> p a f", p=128))

            with tc.tile_pool(name="pp", bufs=20) as pp, \
                 tc.tile_pool(name="rp", bufs=4) as rp, \
                 tc.tile_pool(name="ev", bufs=4) as ev, \
                 tc.tile_pool(name="psB", bufs=3, space="PSUM") as psB:
                # qc-major unit order (pairs inner) so all four pairs of a
                # query chunk finish together

                pend_c = []

                def emit_c_mtile(qc, m):
                    ps = psB.tile([128, 512], f32, tag="c", bufs=2, name="cps")
                    for j in range(NPAIR):
                        nc.tensor.matmul(
                            ps[:, :], wpt[:, j, m * 128:(m + 1) * 128],
                            oT[j][:, qc * 512:(qc + 1) * 512],
                            start=(j == 0), stop=(j == NPAIR - 1))
                    sb = ev.tile([128, 512], f32, tag="sb", name="sb")
                    nc.scalar.activation(sb[:, :], ps[:, :],
                                         mybir.ActivationFunctionType.Copy)
                    nc.sync.dma_start(
                        out=yt[m * 128:(m + 1) * 128,
                               qc * 512:(qc + 1) * 512],
                        in_=sb)

                vstate = {"t": 0}

                def emit_v_unit():
                    t = vstate["t"]
                    if t >= nkt:
                        return
                    vstate["t"] += 1
                    ps = psB.tile([128, FS], f32, tag="s", bufs=3, name="vps")
                    for k in range(nck):
                        nc.tensor.matmul(
                            ps[:, :], xts[k][:, t * 128:(t + 1) * 128],
                            wvt[:, k, :],
                            start=(k == 0), stop=(k == nck - 1))
                    nc.scalar.activation(
                        vhat[t][:, :, 0:HD],
                        ps[:, :].rearrange("p (h d) -> p h d", h=HPC),
                        mybir.ActivationFunctionType.Copy)
                    nc.vector.tensor_copy(vhat[t][:, :, HD], ones_f[:, :])

                def emit_s_step(st):
                    """Emit one S^T + exp + mask step; returns False when done.

                    Both head parities share one 2-bank PSUM tile [128,2,512]
                    so a single 1024-wide exp drains them (halves ACT
                    instruction overhead vs one exp per parity)."""
                    p, qc, kts, i = st["p"], st["qc"], st["kts"], st["i"]
                    if i >= len(kts):
                        return False
                    kt = kts[i]
                    ksl = slice(kt * 128, (kt + 1) * 128)
                    diag = (kt // 4 == qc)
                    off = 128 * (kt % 4) if diag else 0
                    qs2 = slice(qc * 512 + off, (qc + 1) * 512)
                    ps = psB.tile([128, 2, 512], f32, tag="s", bufs=3, name="s")
                    for par in range(2):   # head parity: partitions 0/64
                        row = slice(64 * par, 64 * par + 64)
                        nc.tensor.matmul(
                            ps[:, par, off:512], kT[p][row, ksl],
                            qT[p][row, qs2], start=True, stop=True)
                    pr = pp.tile([128, 2, 512], bf16, tag="P", name="P")
                    nc.scalar.activation(pr[:, :, off:512], ps[:, :, off:512],
                                         EXP, scale=SCALE)
                    if diag:  # mask the 128-wide diagonal strip (both pars)
                        nc.vector.tensor_mul(
                            pr[:, :, off:off + 128],
                            pr[:, :, off:off + 128],
                            mkt[:, None, :].broadcast_to([128, 2, 128]))
                    st["ptiles"].append((pr, off))
                    st["i"] += 1
                    return True

                def emit_pv_chunk(st, n=4):
                    """Emit up to n PV key tiles as same-PSUM-bank matmul
                    runs (all n for parity 0, then all n for parity 1).
                    Consecutive matmuls into one bank avoid the per-MM
                    bank-switch micro-idle penalty. Returns False when done."""
                    p, kts, j0 = st["p"], st["kts"], st["j"]
                    if j0 >= len(kts):
                        return False
                    if st["po"] is None:   # lazy: allocate at first PV chunk
                        st["po"] = [psB.tile([128, 512], f32, tag="po",
                                             bufs=2, name="po")
                                    for _ in range(2)]
                    j1 = min(j0 + n, len(kts))
                    for par in range(2):
                        for j in range(j0, j1):
                            kt = kts[j]
                            pr, off = st["ptiles"][j]
                            nc.tensor.matmul(
                                st["po"][par][0:65, off:512],
                                vhat[kt][:, 2 * p + par, :],
                                pr[:, par, off:512],
                                start=(kt == 0), stop=(kt == kts[-1]))
                    st["j"] = j1
                    return True

                def emit_norm(st):
                    """Fused evacuate+normalize: oT = po[0:64] * (1/po[64]).

                    The den row is evacuated by ACT (fast PSUM port, idle
                    capacity), partition-broadcast by a K=1 matmul against a
                    ones column (213ns on PE), and DVE only runs two
                    full-lane ops (reciprocal + fused multiply) — no
                    single-partition crawls or stream_shuffle."""
                    p, qc = st["p"], st["qc"]
                    qsl = slice(qc * 512, (qc + 1) * 512)
                    for par in range(2):
                        po = st["po"][par]
                        ob = rp.tile([65, 512], bf16, tag="ob", name="ob")
                        nc.scalar.activation(
                            ob[:, :], po[0:65, :],
                            mybir.ActivationFunctionType.Copy)
                        bci = bcis[par]
                        nc.vector.tensor_copy(bci[0:1, :], ob[64:65, :])
                        nc.vector.tensor_copy(bci[32:33, :], ob[64:65, :])
                        bc = rp.tile([64, 512], bf16, tag="bc", name="bc")
                        nc.vector.stream_shuffle(bc[:, :], bci[:, :], [0] * 32)
                        rden = rp.tile([64, 512], bf16, tag="rden", name="rden")
                        with nc.allow_low_precision(
                                reason="bf16 softmax denominators"):
                            nc.vector.reciprocal(rden[:, :], bc[:, :])
                        nc.vector.tensor_mul(
                            oT[p][64 * par:64 * par + 64, qsl],
                            ob[0:64, :], rden[:, :])

                def new_state(p, qc):
                    kts = list(range(min(nkt, 4 * (qc + 1))))
                    return {"p": p, "qc": qc, "kts": kts, "i": 0, "j": 0,
                            "ptiles": [], "po": None}

                def retire(st):
                    while emit_pv_chunk(st):
                        pass
                    if do_norm:
                        emit_norm(st)
                        if 'C' in stages and st["p"] == NPAIR - 1:
                            pend_c.extend(
                                (st["qc"], m) for m in range(nmt))

                # software pipeline: S-phase of unit u interleaved with
                # PV-phase of unit u-1; when u-1's PV is exhausted it is
                # retired immediately (norm emitted early so its PSUM banks
                # recycle)
                if units:
                    for _ in range(4):   # PV of the first unit needs vhat[0..3]
                        emit_v_unit()
                prev = None
                for (p, qc) in units:
                    emit_v_unit()        # one V tile per unit until done
                    cur = new_state(p, qc)
                    more_s = True
                    while more_s:
                        more_s = emit_s_step(cur)
                        if s_only:
                            continue
                        if prev is not None and cur["i"] % 4 == 0:
                            emit_pv_chunk(prev)
                            if prev["j"] >= len(prev["kts"]):
                                retire(prev)
                                prev = None
                    if s_only:
                        continue
                    if prev is not None:
                        retire(prev)
                        prev = None
                    prev = cur
                if prev is not None and not s_only:
                    retire(prev)

            # C post-stage: psB closed, own 4-deep PSUM rotation
            if 'C' in stages and units and do_norm:
                with tc.tile_pool(name="ev2", bufs=4) as ev2, \
                     tc.tile_pool(name="psC", bufs=4, space="PSUM") as psC:
                    for qc in range(nqc):
                        for m in range(nmt):
                            ps = psC.tile([128, 512], f32, tag="c", name="cps")
                            for j in range(NPAIR):
                                nc.tensor.matmul(
                                    ps[:, :], wpt[:, j, m * 128:(m + 1) * 128],
                                    oT[j][:, qc * 512:(qc + 1) * 512],
                                    start=(j == 0), stop=(j == NPAIR - 1))
                            sb = ev2.tile([128, 512], f32, tag="sb", name="sb")
                            nc.scalar.activation(
                                sb[:, :], ps[:, :],
                                mybir.ActivationFunctionType.Copy)
                            nc.sync.dma_start(
                                out=yt[m * 128:(m + 1) * 128,
                                       qc * 512:(qc + 1) * 512],
                                in_=sb)
    nc.compile()
    return nc


def _make_masks():
    import ml_dtypes
    k = np.arange(128)[:, None]
    q = np.arange(128)[None, :]
    return (q >= k).astype(ml_dtypes.bfloat16)


_NC_CACHE = {}


def _get_nc(tpc=T):
    if tpc not in _NC_CACHE:
        _NC_CACHE[tpc] = build_nc(tpc)
    return _NC_CACHE[tpc]


def make_in_maps(x, w_attn, w_proj):
    import ml_dtypes
    bf = ml_dtypes.bfloat16
    masks = _make_masks()
    in_maps = []
    for core in range(N_CORES):
        b, hh = core // 2, core % 2
        s = slice(hh * FS, (hh + 1) * FS)
        in_maps.append({
            "xt": np.ascontiguousarray(np.asarray(x[b]).T).astype(bf),
            "wq": np.ascontiguousarray(w_attn[:, s]).astype(bf),
            "wk": np.ascontiguousarray(w_attn[:, C:][:, s]).astype(bf),
            "wv": np.ascontiguousarray(w_attn[:, 2 * C:][:, s]).astype(bf),
            "wp": np.ascontiguousarray(w_proj[hh * FS:(hh + 1) * FS, :]).astype(bf),
            "mk": masks,
        })
    return in_maps


def kernel(x, w_attn, w_proj):
    nc = _get_nc(T)
    in_maps = make_in_maps(x, w_attn, w_proj)
    res = run_bass_kernel_spmd(nc, in_maps, list(range(N_CORES)))
    y = np.empty((B, T, C), np.float32)
    for b in range(B):
        yt = res.results[2 * b]["yt"] + res.results[2 * b + 1]["yt"]
        y[b] = yt.T
    return y


# revision 35
# speedup vs baseline: 1.2561x; 1.0086x over previous
"""Causal multi-head attention on 8 TRN2 NeuronCores.

Problem: x[4,2048,1024], w_attn[1024,3072], w_proj[1024,1024],
16 heads x 64 dim, causal softmax(QK^T/8)V then output projection.

Sharding: 4-way batch x 2-way head-half. Core c handles batch c//2 and
heads (c%2)*8 .. (c%2)*8+8. Each core computes a partial y^T (its head
half's contribution to the output projection); the host sums the two
partials per batch and transposes.

Per-core layout strategy (all matmuls bf16, cost ~= moving-free-dim):
 - host feeds x^T [1024, 2048] (c_in-major)
 - QKV projection: Q^T,K^T computed feature-major [512, T]; V computed
   token-major [T, 512] (so no on-device transposes anywhere)
 - attention computed transposed: S^T[k,q] = (K^T).T-slices @ Q^T with
   K=64 contraction as two 64-row-base matmuls (head parity 0/64)
 - P = exp(S^T/8) on ACT straight out of PSUM (bf16 output, one
   1024-wide exp covers both parities); causal handling: sub-diagonal
   chunks skipped, diagonal chunks computed and accumulated only on
   their valid column range [off:512], 128-wide strip masked on DVE
 - PV: O^T[d,q] accumulated over key tiles with stationary [V_h | 1]
   (65 cols); PSUM row 64 carries the softmax denominators for free;
   PV emitted in 4-key-tile same-PSUM-bank runs per parity
 - normalize: ONE ACT copy evacuates [O^T | den] (rows 0..64) to SBUF
   (ACT is the only engine with a fast PSUM port: measured DVE PSUM
   reads are ~5x slower on HW than modeled), then DVE works all-SBUF:
   den into rows 0/32 of a bci tile, stream_shuffle broadcast,
   full-lane reciprocal, and a 4x-mode bf16 multiply into oT
 - projection: y^T partial = w_proj_slice.T-rows @ O^T as a post-stage
   with its own 4-deep PSUM rotation; ACT evac, then DMA out
 - software pipeline: S^T/exp of unit i interleaved with PV chunks of
   unit i-1 so the in-order PE stream always has matmul work while ACT
   chews through the exps; po banks allocated lazily at first PV use
 - PSUM budget (stage B): S 3x2 banks + po 2 banks = 8; measured on
   HW that S needs 3 slots of slack or sem latency stalls the PE
"""

import numpy as np
from contextlib import ExitStack

import concourse.bass as bass
import concourse.tile as tile
from concourse import bacc, mybir
from concourse.bass_utils import run_bass_kernel_spmd

f32 = mybir.dt.float32
bf16 = mybir.dt.bfloat16
EXP = mybir.ActivationFunctionType.Exp
MUL = mybir.AluOpType.mult

B, T, C = 4, 2048, 1024
N_HEAD, HD = 16, 64
HPC = 8            # heads per core
FS = HPC * HD      # 512: per-core feature slice for each of q/k/v
NPAIR = HPC // 2   # 4 head pairs
SCALE = 1.0 / 8.0  # 1/sqrt(64)
N_CORES = 8


def build_nc(tpc=T, loop_n=1, dyn_loop=0, stages='ABC'):
    """Build the single-core Bass program (SPMD: same program all cores)."""
    nck = C // 128          # 8 c_in tiles
    nkt = tpc // 128        # key tiles
    nqc = tpc // 512        # query chunks (512 wide)
    nmt = C // 128          # 8 output-channel tiles

    nc = bacc.Bacc("TRN2", target_bir_lowering=False)
    xt = nc.dram_tensor("xt", [C, tpc], bf16, kind="ExternalInput")
    wq = nc.dram_tensor("wq", [C, FS], bf16, kind="ExternalInput")
    wk = nc.dram_tensor("wk", [C, FS], bf16, kind="ExternalInput")
    wv = nc.dram_tensor("wv", [C, FS], bf16, kind="ExternalInput")
    wp = nc.dram_tensor("wp", [FS, C], bf16, kind="ExternalInput")
    mk = nc.dram_tensor("mk", [128, 128], bf16, kind="ExternalInput")
    yt = nc.dram_tensor("yt", [C, tpc], f32, kind="ExternalOutput")

    with tile.TileContext(nc) as tc, ExitStack() as _dl:
     if dyn_loop:
        _dl.enter_context(tc.For_i(0, dyn_loop, 1))
     for _rep in range(loop_n):
      with ExitStack() as stk:
            # tensors that live across stages
            persist = stk.enter_context(tc.tile_pool(name="persist", bufs=1))
            qT = [persist.tile([128, tpc], bf16, tag=f"qT{p}", name=f"qT{p}") for p in range(NPAIR)]
            kT = [persist.tile([128, tpc], bf16, tag=f"kT{p}", name=f"kT{p}") for p in range(NPAIR)]
            # vhat[kt]: [128 keys, 8 heads, 64 dims + ones column]
            vhat = [persist.tile([128, HPC, 65], bf16, tag=f"vh{t}", name=f"vh{t}")
                    for t in range(nkt)]
            mkt = persist.tile([128, 128], bf16, tag="mk")
            nc.sync.dma_start(out=mkt, in_=mk[:, :])
            ones_f = persist.tile([128, HPC], f32, tag="ones")
            nc.vector.memset(ones_f[:, :], 1.0)
            # persistent shuffle inputs (per parity): only rows 0/32 rewritten
            bcis = []
            for bi in range(2):
                b_ = persist.tile([64, 512], bf16, tag=f"bci{bi}", name=f"bci{bi}")
                nc.vector.memset(b_[:, :], 0.0)
                bcis.append(b_)

            # ---------------- Stage A: Q/K projection ----------------
            xa = stk.enter_context(tc.tile_pool(name="xa", bufs=1))
            wb = stk.enter_context(tc.tile_pool(name="wb", bufs=1))
            with tc.tile_pool(name="wa", bufs=1) as wa, \
                 tc.tile_pool(name="psa", bufs=4, space="PSUM") as psa:
                # x^T split across 8 DMAs -> 8 parallel HW queues
                xall = xa.tile([128, nck, tpc], bf16, tag="x")
                xsrc = xt.rearrange("(a p) f -> p a f", p=128)
                for i in range(nck):   # 8 DMAs -> 8 parallel HW queues
                    nc.sync.dma_start(out=xall[:, i, :], in_=xsrc[:, i, :])
                xts = [xall[:, i, :] for i in range(nck)]

                # Q^T / K^T feature-major: out[feat, tok]; one DMA per weight
                wts = {}
                for nm, wsrc in (("q", wq), ("k", wk)):
                    wt = wa.tile([128, nck, FS], bf16, tag=f"w{nm}", name=f"wt{nm}")
                    nc.sync.dma_start(
                        out=wt, in_=wsrc.rearrange("(a p) f -> p a f", p=128))
                    wts[nm] = wt
                nw = tpc // 512            # psum bank limits N*f32 to 512
                wd = tpc // nw
                for dst, nm in ((qT, "q"), (kT, "k")):
                    for m in range(NPAIR):
                        wt = wts[nm]
                        for n in range(nw):
                            ps = psa.tile([128, wd], f32, tag="ps")
                            for k in range(nck):
                                nc.tensor.matmul(
                                    ps[:, :], wt[:, k, m * 128:(m + 1) * 128],
                                    xts[k][:, n * wd:(n + 1) * wd],
                                    start=(k == 0), stop=(k == nck - 1))
                            nc.scalar.activation(
                                dst[m][:, n * wd:(n + 1) * wd], ps[:, :],
                                mybir.ActivationFunctionType.Copy)

                # V weights loaded here; V compute is interleaved into B
                wvt = wb.tile([128, nck, FS], bf16, tag="wv")
                nc.sync.dma_start(out=wvt, in_=wv.rearrange("(a p) f -> p a f", p=128))

            # ---------------- Stages B+C ----------------
            otp = stk.enter_context(tc.tile_pool(name="ot", bufs=1))
            oT = [otp.tile([128, tpc], bf16, tag=f"oT{p}", name=f"oT{p}") for p in range(NPAIR)]

            wc = stk.enter_context(tc.tile_pool(name="wc", bufs=1))
            units = ([(p, qc) for qc in range(nqc) for p in range(NPAIR)]
                     if ('B' in stages or 'S' in stages or 'P' in stages)
                     else [])
            s_only = ('B' not in stages and 'P' not in stages)
            do_norm = 'B' in stages      # 'P': S+PV but no norm/C
            wpt = None
            if 'C' in stages and units:
                wpt = wc.tile([128, NPAIR, C], bf16, tag="wp", name="wpt")
                nc.sync.dma_start(
                    out=wpt, in_=wp.rearrange("(a p) f -> p a f", p=128))

            with tc.tile_pool(name="pp", bufs=26) as pp, \
                 tc.tile_pool(name="rp", bufs=4) as rp, \
                 tc.tile_pool(name="ev", bufs=4) as ev, \
                 tc.tile_pool(name="psB", bufs=3, space="PSUM") as psB:
                # qc-major unit order (pairs inner) so all four pairs of a
                # query chunk finish together

                pend_c = []

                def emit_c_mtile(qc, m):
                    ps = psB.tile([128, 512], f32, tag="c", bufs=2, name="cps")
                    for j in range(NPAIR):
                        nc.tensor.matmul(
                            ps[:, :], wpt[:, j, m * 128:(m + 1) * 128],
                            oT[j][:, qc * 512:(qc + 1) * 512],
                            start=(j == 0), stop=(j == NPAIR - 1))
                    sb = ev.tile([128, 512], f32, tag="sb", name="sb")
                    nc.scalar.activation(sb[:, :], ps[:, :],
                                         mybir.ActivationFunctionType.Copy)
                    nc.sync.dma_start(
                        out=yt[m * 128:(m + 1) * 128,
                               qc * 512:(qc + 1) * 512],
                        in_=sb)

                vstate = {"t": 0}

                def emit_v_unit():
                    t = vstate["t"]
                    if t >= nkt:
                        return
                    vstate["t"] += 1
                    ps = psB.tile([128, FS], f32, tag="s", bufs=3, name="vps")
                    for k in range(nck):
                        nc.tensor.matmul(
                            ps[:, :], xts[k][:, t * 128:(t + 1) * 128],
                            wvt[:, k, :],
                            start=(k == 0), stop=(k == nck - 1))
                    nc.scalar.activation(
                        vhat[t][:, :, 0:HD],
                        ps[:, :].rearrange("p (h d) -> p h d", h=HPC),
                        mybir.ActivationFunctionType.Copy)
                    nc.vector.tensor_copy(vhat[t][:, :, HD], ones_f[:, :])

                def emit_s_step(st):
                    """Emit one S^T + exp + mask step; returns False when done.

                    Both head parities share one 2-bank PSUM tile [128,2,512]
                    so a single 1024-wide exp drains them (halves ACT
                    instruction overhead vs one exp per parity)."""
                    p, qc, kts, i = st["p"], st["qc"], st["kts"], st["i"]
                    if i >= len(kts):
                        return False
                    kt = kts[i]
                    ksl = slice(kt * 128, (kt + 1) * 128)
                    diag = (kt // 4 == qc)
                    off = 128 * (kt % 4) if diag else 0
                    qs2 = slice(qc * 512 + off, (qc + 1) * 512)
                    ps = psB.tile([128, 2, 512], f32, tag="s", bufs=3, name="s")
                    for par in range(2):   # head parity: partitions 0/64
                        row = slice(64 * par, 64 * par + 64)
                        nc.tensor.matmul(
                            ps[:, par, off:512], kT[p][row, ksl],
                            qT[p][row, qs2], start=True, stop=True)
                    pr = pp.tile([128, 2, 512], bf16, tag="P", name="P")
                    nc.scalar.activation(pr[:, :, off:512], ps[:, :, off:512],
                                         EXP, scale=SCALE)
                    if diag:  # mask the 128-wide diagonal strip (both pars)
                        nc.vector.tensor_mul(
                            pr[:, :, off:off + 128],
                            pr[:, :, off:off + 128],
                            mkt[:, None, :].broadcast_to([128, 2, 128]))
                    st["ptiles"].append((pr, off))
                    st["i"] += 1
                    return True

                def emit_pv_chunk(st, n=4):
                    """Emit up to n PV key tiles as same-PSUM-bank matmul
                    runs (all n for parity 0, then all n for parity 1).
                    Consecutive matmuls into one bank avoid the per-MM
                    bank-switch micro-idle penalty. Returns False when done."""
                    p, kts, j0 = st["p"], st["kts"], st["j"]
                    if j0 >= len(kts):
                        return False
                    if st["po"] is None:   # lazy: allocate at first PV chunk
                        st["po"] = [psB.tile([128, 512], f32, tag="po",
                                             bufs=2, name="po")
                                    for _ in range(2)]
                    j1 = min(j0 + n, len(kts))
                    for par in range(2):
                        for j in range(j0, j1):
                            kt = kts[j]
                            pr, off = st["ptiles"][j]
                            nc.tensor.matmul(
                                st["po"][par][0:65, off:512],
                                vhat[kt][:, 2 * p + par, :],
                                pr[:, par, off:512],
                                start=(kt == 0), stop=(kt == kts[-1]))
                    st["j"] = j1
                    return True

                def emit_norm(st):
                    """Fused evacuate+normalize: oT = po[0:64] * (1/po[64]).

                    The den row is evacuated by ACT (fast PSUM port, idle
                    capacity), partition-broadcast by a K=1 matmul against a
                    ones column (213ns on PE), and DVE only runs two
                    full-lane ops (reciprocal + fused multiply) — no
                    single-partition crawls or stream_shuffle."""
                    p, qc = st["p"], st["qc"]
                    qsl = slice(qc * 512, (qc + 1) * 512)
                    for par in range(2):
                        po = st["po"][par]
                        ob = rp.tile([65, 512], bf16, tag="ob", name="ob")
                        nc.scalar.activation(
                            ob[:, :], po[0:65, :],
                            mybir.ActivationFunctionType.Copy)
                        bci = bcis[par]
                        nc.vector.tensor_copy(bci[0:1, :], ob[64:65, :])
                        nc.vector.tensor_copy(bci[32:33, :], ob[64:65, :])
                        bc = rp.tile([64, 512], bf16, tag="bc", name="bc")
                        nc.vector.stream_shuffle(bc[:, :], bci[:, :], [0] * 32)
                        rden = rp.tile([64, 512], bf16, tag="rden", name="rden")
                        with nc.allow_low_precision(
                                reason="bf16 softmax denominators"):
                            nc.vector.reciprocal(rden[:, :], bc[:, :])
                        nc.vector.tensor_mul(
                            oT[p][64 * par:64 * par + 64, qsl],
                            ob[0:64, :], rden[:, :])

                def new_state(p, qc):
                    kts = list(range(min(nkt, 4 * (qc + 1))))
                    return {"p": p, "qc": qc, "kts": kts, "i": 0, "j": 0,
                            "ptiles": [], "po": None}

                def retire(st):
                    while emit_pv_chunk(st):
                        pass
                    if do_norm:
                        emit_norm(st)
                        if 'C' in stages and st["p"] == NPAIR - 1:
                            pend_c.extend(
                                (st["qc"], m) for m in range(nmt))

                # software pipeline: S-phase of unit u interleaved with
                # PV-phase of unit u-1; when u-1's PV is exhausted it is
                # retired immediately (norm emitted early so its PSUM banks
                # recycle)
                if units:
                    for _ in range(4):   # PV of the first unit needs vhat[0..3]
                        emit_v_unit()
                prev = None
                for (p, qc) in units:
                    emit_v_unit()        # one V tile per unit until done
                    cur = new_state(p, qc)
                    more_s = True
                    while more_s:
                        more_s = emit_s_step(cur)
                        if s_only:
                            continue
                        if prev is not None and cur["i"] % 4 == 0:
                            emit_pv_chunk(prev)
                            if prev["j"] >= len(prev["kts"]):
                                retire(prev)
                                prev = None
                    if s_only:
                        continue
                    if prev is not None:
                        retire(prev)
                        prev = None
                    prev = cur
                if prev is not None and not s_only:
                    retire(prev)

            # C post-stage: psB closed, own 4-deep PSUM rotation
            if 'C' in stages and units and do_norm:
                with tc.tile_pool(name="ev2", bufs=4) as ev2, \
                     tc.tile_pool(name="psC", bufs=4, space="PSUM") as psC:
                    for qc in range(nqc):
                        for m in range(nmt):
                            ps = psC.tile([128, 512], f32, tag="c", name="cps")
                            for j in range(NPAIR):
                                nc.tensor.matmul(
                                    ps[:, :], wpt[:, j, m * 128:(m + 1) * 128],
                                    oT[j][:, qc * 512:(qc + 1) * 512],
                                    start=(j == 0), stop=(j == NPAIR - 1))
                            sb = ev2.tile([128, 512], f32, tag="sb", name="sb")
                            nc.scalar.activation(
                                sb[:, :], ps[:, :],
                                mybir.ActivationFunctionType.Copy)
                            nc.sync.dma_start(
                                out=yt[m * 128:(m + 1) * 128,
                                       qc * 512:(qc + 1) * 512],
                                in_=sb)
    nc.compile()
    return nc


def _make_masks():
    import ml_dtypes
    k = np.arange(128)[:, None]
    q = np.arange(128)[None, :]
    return (q >= k).astype(ml_dtypes.bfloat16)


_NC_CACHE = {}


def _get_nc(tpc=T):
    if tpc not in _NC_CACHE:
        _NC_CACHE[tpc] = build_nc(tpc)
    return _NC_CACHE[tpc]


def make_in_maps(x, w_attn, w_proj):
    import ml_dtypes
    bf = ml_dtypes.bfloat16
    masks = _make_masks()
    in_maps = []
    for core in range(N_CORES):
        b, hh = core // 2, core % 2
        s = slice(hh * FS, (hh + 1) * FS)
        in_maps.append({
            "xt": np.ascontiguousarray(np.asarray(x[b]).T).astype(bf),
            "wq": np.ascontiguousarray(w_attn[:, s]).astype(bf),
            "wk": np.ascontiguousarray(w_attn[:, C:][:, s]).astype(bf),
            "wv": np.ascontiguousarray(w_attn[:, 2 * C:][:, s]).astype(bf),
            "wp": np.ascontiguousarray(w_proj[hh * FS:(hh + 1) * FS, :]).astype(bf),
            "mk": masks,
        })
    return in_maps


def kernel(x, w_attn, w_proj):
    nc = _get_nc(T)
    in_maps = make_in_maps(x, w_attn, w_proj)
    res = run_bass_kernel_spmd(nc, in_maps, list(range(N_CORES)))
    y = np.empty((B, T, C), np.float32)
    for b in range(B):
        yt = res.results[2 * b]["yt"] + res.results[2 * b + 1]["yt"]
        y[b] = yt.T
    return y
